# revision 10
# baseline (speedup 1.0000x reference)
"""Trainium2 Bass kernel for nn_ObservationTokenizer.

Reference computation (per batch row of encoded_obs [B=8192, 643]):
  - the 643-dim obs vector is a static layout of groups [marker, features...]:
      batt: 20 groups x (1+8)  starting at col 0    (marker == 1001.0)
      ev:   20 groups x (1+12) starting at col 180  (marker == 1002.0)
      sro:  10 groups x (1+16) starting at col 440  (marker == 2001.0)
      nfc:   1 group  x (1+32) starting at col 610  (marker == 3001.0)
  - each group's features are projected to d_model=512 with a per-type
    Linear (W [d,512], b [512])
  - outputs: ca_tokens [B,40,512] (batt then ev), sro_tok [B,10,512],
    nfc_tok [B,1,512]

Strategy (pure data parallel over batch, 8 cores x 1024 rows):
  - load [128, 643] row tiles to SBUF in natural layout (fast DMA)
  - PE-transpose a contiguous 4-group block [128, K<=68] to PSUM
    (partition 0), copy to SBUF
  - one fp32 matmul per token: stationary = the whole transposed block
    [K, 128], moving = a zero-padded weight slab [K, 512] that holds
    [bias/marker; W] at the token's rows (the constant marker column turns
    the bias add into part of the matmul; zero rows are free since the
    matmul streams N=512 columns regardless of K)
  - copy PSUM [128,512] -> SBUF staging (split across DVE and ACT),
    accumulate 5-token chunks, DMA out as ~1.3 MB contiguous transfers.
"""

import numpy as np

# ---- static problem layout (hardcoded; must match the reference) ----
B = 8192
N_CORES = 8
BL = B // N_CORES  # rows per core
OBS_DIM = 643
D_MODEL = 512
P = 128  # partition tile (batch rows per tile)
N_TILES = BL // P  # 8

# (name, start_col, n_groups, group_size(marker+feats), marker_value)
TYPES = [
    ("batt", 0, 20, 9, 1001.0),
    ("ev", 180, 20, 13, 1002.0),
    ("sro", 440, 10, 17, 2001.0),
    ("nfc", 610, 1, 33, 3001.0),
]

_CACHE = {}


def _layout():
    """Blocks of up to 4 consecutive groups transposed together.

    Returns:
      blocks: list of (col0, kblk, [(type_idx, group, j_in_block), ...])
      slabs:  list of (type_idx, j) weight-slab patterns, index = slab id
      slab_of: {(type_idx, j): slab_id}
      where:  {(type_idx, group): (block_id, j_in_block)}
    """
    blocks = []
    where = {}
    slab_of = {}
    slabs = []
    for ti, (_name, start, n, gs, _m) in enumerate(TYPES):
        for g0 in range(0, n, 4):
            ng = min(4, n - g0)
            col0 = start + g0 * gs
            entries = []
            for j in range(ng):
                entries.append((ti, g0 + j, j))
                where[(ti, g0 + j)] = (len(blocks), j)
                if (ti, j) not in slab_of:
                    slab_of[(ti, j)] = len(slabs)
                    slabs.append((ti, j))
            blocks.append((col0, ng * gs, entries))
    return blocks, slabs, slab_of, where


def _build_nc():
    import concourse.mybir as mybir
    import concourse.tile as tile
    from concourse import bacc
    from concourse.masks import make_identity

    f32 = mybir.dt.float32
    nc = bacc.Bacc(
        "TRN2",
        target_bir_lowering=False,
        debug=False,
        num_devices=N_CORES,
        enable_partition_id=False,
    )

    blocks, slabs, slab_of, where = _layout()
    n_slabs = len(slabs)

    enc = nc.dram_tensor("enc", [BL, OBS_DIM], f32, kind="ExternalInput")
    w_dram = nc.dram_tensor("w_all", [n_slabs, 128, D_MODEL], f32,
                            kind="ExternalInput")
    ca = nc.dram_tensor("ca", [BL, 40, D_MODEL], f32, kind="ExternalOutput")
    sro = nc.dram_tensor("sro", [BL, 10, D_MODEL], f32, kind="ExternalOutput")
    nfc = nc.dram_tensor("nfc", [BL, 1, D_MODEL], f32, kind="ExternalOutput")

    # token order inside each output tensor: (out_dram, token_idx, type_idx, group)
    tokens = []
    for ti, (name, _s, n, _gs, _m) in enumerate(TYPES):
        for g in range(n):
            if name == "batt":
                tokens.append((ca, g, ti, g))
            elif name == "ev":
                tokens.append((ca, 20 + g, ti, g))
            elif name == "sro":
                tokens.append((sro, g, ti, g))
            else:
                tokens.append((nfc, 0, ti, g))
    # chunks of up to 5 consecutive tokens of the same output tensor
    chunks = []
    for out_t, ntok in ((ca, 40), (sro, 10), (nfc, 1)):
        toks = sorted((t for t in tokens if t[0] is out_t), key=lambda t: t[1])
        for c0 in range(0, ntok, 5):
            chunks.append((out_t, toks[c0 : c0 + 5]))

    with tile.TileContext(nc) as tc:
        with (
            tc.tile_pool(name="const", bufs=1) as const_pool,
            tc.tile_pool(name="inp", bufs=3) as in_pool,
            tc.tile_pool(name="tsb", bufs=2) as t_pool,
            tc.tile_pool(name="stage", bufs=6) as stage_pool,
            tc.tile_pool(name="tp", bufs=3, space="PSUM") as tp_pool,
            tc.tile_pool(name="po", bufs=4, space="PSUM") as po_pool,
        ):
            ident = const_pool.tile([128, 128], f32)
            make_identity(nc, ident)
            w_all = const_pool.tile([128, n_slabs * D_MODEL], f32)
            # single DMA for all weight slabs (keeps matmul wait fan-in low)
            nc.gpsimd.dma_start(
                w_all[:].rearrange("p (i d) -> p i d", d=D_MODEL),
                w_dram[:].rearrange("i p d -> p i d"),
            )

            # strict even/odd DVE/ACT alternation; with po bufs=4 each po
            # slot is always drained by the same engine (1 WAR wait)
            ncopy = 0

            def copy(out_ap, in_ap):
                nonlocal ncopy
                if ncopy % 2 == 1:
                    nc.scalar.copy(out_ap, in_ap)
                else:
                    nc.vector.tensor_copy(out_ap, in_ap)
                ncopy += 1

            for t in range(N_TILES):
                et = in_pool.tile([128, OBS_DIM], f32, tag="et")
                nc.gpsimd.dma_start(et[:], enc[t * P : (t + 1) * P, :])

                tsb = t_pool.tile([128, len(blocks) * 128], f32, tag="tsb")
                for bi, (col0, kblk, _entries) in enumerate(blocks):
                    tp = tp_pool.tile([128, 128], f32, tag="tp",
                                      name=f"tp_{t}_{bi}")
                    nc.tensor.transpose(
                        tp[:kblk, :], et[:, col0 : col0 + kblk], ident[:]
                    )
                    # tsb copies pinned to DVE: matmuls then RAW-wait on a
                    # single engine's tick
                    nc.vector.tensor_copy(
                        tsb[:kblk, bi * 128 : (bi + 1) * 128], tp[:kblk, :]
                    )

                for out_t, toks in chunks:
                    k = len(toks)
                    if out_t is nfc:
                        st = stage_pool.tile([128, D_MODEL], f32,
                                             tag="st_nfc", bufs=2)
                    else:
                        st = stage_pool.tile([128, 5 * D_MODEL], f32, tag="st")
                    for j, (_o, _tok_i, ti, g) in enumerate(toks):
                        name, _start, _n, gs, _m = TYPES[ti]
                        bi, jj = where[(ti, g)]
                        kblk = blocks[bi][1]
                        si = slab_of[(ti, jj)]
                        po = po_pool.tile([128, D_MODEL], f32, tag="po",
                                          name=f"po_{t}_{ti}_{g}")
                        nc.tensor.matmul(
                            po[:],
                            tsb[:kblk, bi * 128 : (bi + 1) * 128],
                            w_all[:kblk, si * D_MODEL : (si + 1) * D_MODEL],
                        )
                        copy(st[:, j * D_MODEL : (j + 1) * D_MODEL], po[:])
                    tok0 = toks[0][1]
                    nc.sync.dma_start(
                        out_t[t * P : (t + 1) * P, tok0 : tok0 + k, :],
                        st[:, : k * D_MODEL].rearrange(
                            "p (n d) -> p n d", d=D_MODEL
                        ),
                    )
    nc.compile()
    return nc


def _prep_weights(inputs):
    """[n_slabs, 128, 512]: slab (ti, j) holds [b/marker; W] at rows
    gs*j..gs*j+gs, zeros elsewhere."""
    _blocks, slabs, _slab_of, _where = _layout()
    out = np.zeros((len(slabs), 128, D_MODEL), dtype=np.float32)
    for si, (ti, j) in enumerate(slabs):
        name, _start, _n, gs, marker = TYPES[ti]
        W = np.asarray(inputs[f"W_{name}"], np.float32)
        b = np.asarray(inputs[f"b_{name}"], np.float32)
        blk = np.concatenate([(b / np.float32(marker))[None, :], W], axis=0)
        assert blk.shape == (gs, D_MODEL)
        out[si, gs * j : gs * j + gs] = blk
    return out


def _make_in_maps(inputs):
    enc = np.ascontiguousarray(inputs["encoded_obs"], dtype=np.float32)
    assert enc.shape == (B, OBS_DIM)
    w_all = _prep_weights(inputs)
    return [
        {
            "enc": np.ascontiguousarray(enc[c * BL : (c + 1) * BL]),
            "w_all": w_all,
        }
        for c in range(N_CORES)
    ]


def _run(inputs, trace=False):
    from concourse.bass_utils import run_bass_kernel_spmd

    if "nc" not in _CACHE:
        _CACHE["nc"] = _build_nc()
    nc = _CACHE["nc"]

    in_maps = _make_in_maps(inputs)
    res = run_bass_kernel_spmd(
        nc, in_maps, core_ids=list(range(N_CORES)), trace=trace
    )
    ca = np.concatenate([r["ca"] for r in res.results], axis=0)
    sro = np.concatenate([r["sro"] for r in res.results], axis=0)
    nfc = np.concatenate([r["nfc"] for r in res.results], axis=0)
    return (ca, sro, nfc), res


def kernel(encoded_obs, W_batt, b_batt, W_ev, b_ev, W_sro, b_sro, W_nfc, b_nfc):
    out, _res = _run(
        dict(
            encoded_obs=encoded_obs,
            W_batt=W_batt, b_batt=b_batt,
            W_ev=W_ev, b_ev=b_ev,
            W_sro=W_sro, b_sro=b_sro,
            W_nfc=W_nfc, b_nfc=b_nfc,
        )
    )
    return out
